# revision 13
# baseline (speedup 1.0000x reference)
"""Trainium2 Bass kernel for nn_ClassificationRNN2 (embedding + LSTM + ragged attention + head).

Strategy: data-parallel over batch across 8 NeuronCores (64 samples/core),
weights/embedding replicated, no collectives.

Host/runtime: the axon tunnel runs at ~35MB/s, so the warm-call cost is all
input transfer. The jitted shard_map executable is cached across calls and
the weight/embedding tensors live device-resident (refreshed only when a
content fingerprint changes, verified concurrently with the dispatch). Each
call ships one small i32 buffer per core (token tiles + sorted lens, ~77KB);
all ragged-length-dependent addressing (q/M gather offsets, softmax mask) is
derived from lens on device in exact f32.

Per-core layout: "transposed" H-major state. Per step t:
  g^T[1024,64] = Wcat^T.T @ [x_t; h_{t-1}]  (24 bf16 matmuls, fp32 PSUM)
  gates on ACT (sigmoid/tanh share one table set), cell update on DVE,
  h_t transposed (PE) to b-major and stored to a DRAM scratch [BC,T,H].
Attention reads that scratch: q via indirect gather at len-1, the ragged
reshape-view M[b] = flat_b.reshape(H, len_b) via indirect gather with
offsets h*len_b, score/ctx via per-sample matmuls.
"""

import numpy as np
import ml_dtypes

import concourse.bass as bass
import concourse.mybir as mybir
import concourse.tile as tile
from concourse import bacc
from concourse.bass import IndirectOffsetOnAxis
from concourse.masks import make_identity

BF16 = mybir.dt.bfloat16
F32 = mybir.dt.float32
I32 = mybir.dt.int32
AF = mybir.ActivationFunctionType
ALU = mybir.AluOpType
AX = mybir.AxisListType

NCORES = 8
B, L, D, H, V, C = 512, 300, 128, 256, 100001, 14
G = 4 * H  # 1024 gate dims


def build_kernel(BC, T, VV, CH_STEPS, enable_asserts=False):
    """Per-core program. BC=batch/core, T=steps, VV=vocab rows,
    CH_STEPS*BC must be a multiple of 128 and divide BC*T."""
    TOK = BC * T
    TOK_CH = BC * CH_STEPS
    assert TOK_CH % 128 == 0 and TOK % TOK_CH == 0
    TPC = TOK_CH // 128          # 128-token transpose tiles per chunk
    NCH = TOK // TOK_CH          # x^T chunks
    PW = ((T + 127) // 128) * 128
    NK = PW // 128               # l-chunks for ctx
    LCH = [min(128, T - k * 128) for k in range(NK)]

    nc = bacc.Bacc("TRN2", target_bir_lowering=False, debug=False,
                   enable_asserts=enable_asserts)

    # ---- DRAM I/O ----
    emb_d = nc.dram_tensor("emb", [VV, D], BF16, kind="ExternalInput")
    wt_d = nc.dram_tensor("wt", [3, 128, G], BF16, kind="ExternalInput")
    w1t_d = nc.dram_tensor("w1t", [4, 128, H], BF16, kind="ExternalInput")
    w2t_d = nc.dram_tensor("w2t", [2, 128, C], BF16, kind="ExternalInput")
    biasg_d = nc.dram_tensor("biasg", [128, 8], F32, kind="ExternalInput")
    b1t_d = nc.dram_tensor("b1t", [128, 2], F32, kind="ExternalInput")
    b2c_d = nc.dram_tensor("b2c", [C, 1], F32, kind="ExternalInput")
    NT = TOK // 128              # total 128-token tiles
    # single per-call tensor: cols 0..NT-1 token tiles, col NT = sorted lens
    dynbuf_d = nc.dram_tensor("dynbuf", [128, NT + 1], I32, kind="ExternalInput")
    # static iota/address-helper constants (uploaded once with the weights)
    iotal_d = nc.dram_tensor("iotal", [BC, T], F32, kind="ExternalInput")
    bth_d = nc.dram_tensor("bth", [128, BC], F32, kind="ExternalInput")
    iotap_d = nc.dram_tensor("iotap", [128, 2], F32, kind="ExternalInput")
    qbias_d = nc.dram_tensor("qbias", [BC, 1], F32, kind="ExternalInput")
    eye_d = nc.dram_tensor("eye", [1, BC * BC], F32, kind="ExternalInput")
    out_d = nc.dram_tensor("out", [BC, C], F32, kind="ExternalOutput")
    # internal DRAM scratch: per-sample row-major hidden states, flat for gathers
    hs_d = nc.dram_tensor("hsflat", [BC * T * H, 1], BF16)
    hs3 = hs_d[:].rearrange("(b t h) one -> b t (h one)", b=BC, t=T)

    with tile.TileContext(nc) as tc:
        with tc.tile_pool(name="persist", bufs=1) as pp:
            # ---- persistent SBUF ----
            idf = pp.tile([128, 128], F32, tag="idf")
            make_identity(nc, idf[:])
            idb = pp.tile([128, 128], BF16, tag="idb")
            nc.vector.tensor_copy(idb[:], idf[:])

            w_sb = pp.tile([128, 3 * G], BF16, tag="w")
            w1_sb = pp.tile([128, 4 * H], BF16, tag="w1")
            w2_sb = pp.tile([128, 2 * C], BF16, tag="w2")
            for k in range(3):
                nc.sync.dma_start(w_sb[:, k * G:(k + 1) * G], wt_d[k])
            for k in range(4):
                nc.sync.dma_start(w1_sb[:, k * H:(k + 1) * H], w1t_d[k])
            for k in range(2):
                nc.sync.dma_start(w2_sb[:, k * C:(k + 1) * C], w2t_d[k])
            bg_sb = pp.tile([128, 8], F32, tag="bg")
            nc.sync.dma_start(bg_sb[:], biasg_d[:])
            b1_sb = pp.tile([128, 2], F32, tag="b1")
            nc.sync.dma_start(b1_sb[:], b1t_d[:])
            b2_sb = pp.tile([C, 1], F32, tag="b2")
            nc.sync.dma_start(b2_sb[:], b2c_d[:])

            xT = [pp.tile([128, TOK_CH], BF16, tag=f"xT{c}", name=f"xT{c}")
                  for c in range(NCH)]

            # ========== phase 0: ragged address math from lens ==========
            # dynbuf carries tokens + sorted lens; derive qoff/moff/mask here
            # (f32 is exact: all values < 2^24).
            idx_all = pp.tile([128, NT + 1], I32, tag="idx")
            nc.sync.dma_start(idx_all[:], dynbuf_d[:])
            iotal_sb = pp.tile([BC, T], F32, tag="iotal")
            nc.sync.dma_start(iotal_sb[:], iotal_d[:])
            bth_sb = pp.tile([128, BC], F32, tag="bth")
            nc.sync.dma_start(bth_sb[:], bth_d[:])
            iotap_sb = pp.tile([128, 2], F32, tag="iotap")
            nc.sync.dma_start(iotap_sb[:], iotap_d[:])
            qbias_sb = pp.tile([BC, 1], F32, tag="qbias")
            nc.sync.dma_start(qbias_sb[:], qbias_d[:])
            ones_sb = pp.tile([1, 128], F32, tag="ones")
            nc.vector.memset(ones_sb[:], 1.0)

            lens_f = pp.tile([BC, 1], F32, tag="lensf")
            nc.vector.tensor_copy(lens_f[:], idx_all[:BC, NT:NT + 1])
            moff = pp.tile([128, 2 * BC], I32, tag="moff")
            qoff = pp.tile([BC, 1], I32, tag="qoff")
            mask = pp.tile([BC, T], F32, tag="mask")
            qf = pp.tile([BC, 1], F32, tag="qf")
            # qoff = lens*H + (b*T*H - H)
            nc.vector.tensor_scalar(qf[:], lens_f[:], float(H),
                                    scalar2=qbias_sb[:, 0:1],
                                    op0=ALU.mult, op1=ALU.add)
            nc.vector.tensor_copy(qoff[:], qf[:])
            # mask = (l >= lens) * -1e30
            nc.vector.tensor_scalar(mask[:], iotal_sb[:], lens_f[:, 0:1],
                                    scalar2=-1e30,
                                    op0=ALU.is_ge, op1=ALU.mult)
            with tc.tile_pool(name="p0", bufs=1, space="PSUM") as ps0:
                # lens broadcast over partitions: lensB[p,b] = lens[b]
                lrp = ps0.tile([1, BC], F32, tag="lrp")
                nc.tensor.matmul(out=lrp[:], lhsT=lens_f[:], rhs=idf[:BC, :BC],
                                 start=True, stop=True)
                lens_row = pp.tile([1, BC], F32, tag="lensrow")
                nc.vector.tensor_copy(lens_row[:], lrp[:])
                lBp = ps0.tile([128, BC], F32, tag="lBp")
                nc.tensor.matmul(out=lBp[:], lhsT=ones_sb[:], rhs=lens_row[:],
                                 start=True, stop=True)
                lensB = pp.tile([128, BC], F32, tag="lensB")
                nc.vector.tensor_copy(lensB[:], lBp[:])
            # moff[u][p,b] = (u*128+p)*lens[b] + b*T*H
            mf = pp.tile([128, 2 * BC], F32, tag="mf")
            for u in range(2):
                cs = slice(u * BC, (u + 1) * BC)
                nc.vector.tensor_scalar(mf[:, cs], lensB[:], iotap_sb[:, u:u + 1],
                                        scalar2=None, op0=ALU.mult)
                nc.vector.tensor_tensor(out=mf[:, cs], in0=mf[:, cs],
                                        in1=bth_sb[:], op=ALU.add)
            nc.vector.tensor_copy(moff[:], mf[:])

            # ========== phase 1: embedding gather + transpose to x^T ==========
            # HW indirect DMA consumes ONE offset per dest partition; the whole
            # per-partition free extent streams contiguously from it. So gather
            # one 128-token tile ([128, D]) per instruction.
            with tc.tile_pool(name="gat", bufs=4) as gp, \
                 tc.tile_pool(name="ps1", bufs=2, space="PSUM") as ps1:
                for g in range(NT):
                    ci, j = g // TPC, g % TPC
                    xrows = gp.tile([128, 128], BF16, tag="xrows")
                    nc.gpsimd.indirect_dma_start(
                        out=xrows[:], out_offset=None,
                        in_=emb_d[:],
                        in_offset=IndirectOffsetOnAxis(ap=idx_all[:, g:g + 1],
                                                       axis=0),
                    )
                    trp = ps1.tile([128, 128], BF16, tag="trx")
                    nc.tensor.transpose(out=trp[:], in_=xrows[:],
                                        identity=idb[:])
                    nc.vector.tensor_copy(
                        xT[ci][:, j * 128:(j + 1) * 128], trp[:])

                # ========== phase 2: LSTM recurrence ==========
                with tc.tile_pool(name="st", bufs=1) as sp, \
                     tc.tile_pool(name="lp", bufs=2) as lp, \
                     tc.tile_pool(name="ps2", bufs=2, space="PSUM") as ps2:
                    c_sb = sp.tile([128, 2 * BC], F32, tag="c")
                    nc.gpsimd.memset(c_sb[:], 0.0)
                    hT_prev = lp.tile([128, 2 * BC], BF16, tag="hT")
                    nc.gpsimd.memset(hT_prev[:], 0.0)

                    for t in range(T):
                        ch, col = t // CH_STEPS, (t % CH_STEPS) * BC
                        xcol = xT[ch][:, col:col + BC]
                        gA = ps2.tile([128, 4 * BC], F32, tag="gA")
                        gB = ps2.tile([128, 4 * BC], F32, tag="gB")
                        for j in range(8):
                            out = (gA if j < 4 else gB)[:, (j % 4) * BC:(j % 4 + 1) * BC]
                            wj = slice(j * 128, (j + 1) * 128)
                            nc.tensor.matmul(out=out, lhsT=w_sb[:, wj], rhs=xcol,
                                             start=True, stop=False)
                            nc.tensor.matmul(out=out, lhsT=w_sb[:, G:][:, wj],
                                             rhs=hT_prev[:, :BC], start=False, stop=False)
                            nc.tensor.matmul(out=out, lhsT=w_sb[:, 2 * G:][:, wj],
                                             rhs=hT_prev[:, BC:], start=False, stop=True)
                        # gates: i=j0,1  f=j2,3 (gA)   g~=j4,5  o=j6,7 (gB)
                        i_sb = lp.tile([128, 2 * BC], F32, tag="i")
                        f_sb = lp.tile([128, 2 * BC], F32, tag="f")
                        g_sb = lp.tile([128, 2 * BC], F32, tag="g")
                        o_sb = lp.tile([128, 2 * BC], F32, tag="o")
                        for u in range(2):
                            cs = slice(u * BC, (u + 1) * BC)
                            cs2 = slice(2 * BC + u * BC, 2 * BC + (u + 1) * BC)
                            nc.scalar.activation(i_sb[:, cs], gA[:, cs], AF.Sigmoid,
                                                 bias=bg_sb[:, u:u + 1])
                            nc.scalar.activation(f_sb[:, cs], gA[:, cs2], AF.Sigmoid,
                                                 bias=bg_sb[:, 2 + u:3 + u])
                            nc.scalar.activation(g_sb[:, cs], gB[:, cs], AF.Tanh,
                                                 bias=bg_sb[:, 4 + u:5 + u])
                            nc.scalar.activation(o_sb[:, cs], gB[:, cs2], AF.Sigmoid,
                                                 bias=bg_sb[:, 6 + u:7 + u])
                        t1 = lp.tile([128, 2 * BC], F32, tag="t1")
                        nc.vector.tensor_tensor(out=t1[:], in0=i_sb[:], in1=g_sb[:],
                                                op=ALU.mult)
                        nc.vector.tensor_tensor(out=c_sb[:], in0=c_sb[:], in1=f_sb[:],
                                                op=ALU.mult)
                        nc.vector.tensor_tensor(out=c_sb[:], in0=c_sb[:], in1=t1[:],
                                                op=ALU.add)
                        th = lp.tile([128, 2 * BC], F32, tag="th")
                        nc.scalar.activation(th[:], c_sb[:], AF.Tanh)
                        hT = lp.tile([128, 2 * BC], BF16, tag="hT")
                        nc.vector.tensor_tensor(out=hT[:], in0=o_sb[:], in1=th[:],
                                                op=ALU.mult)
                        # b-major row to DRAM for the attention phase
                        hrow = lp.tile([BC, H], BF16, tag="hrow")
                        for u in range(2):
                            trh = ps2.tile([BC, 128], BF16, tag="trh")
                            nc.tensor.transpose(out=trh[:],
                                                in_=hT[:, u * BC:(u + 1) * BC],
                                                identity=idb[:])
                            nc.vector.tensor_copy(hrow[:, u * 128:(u + 1) * 128],
                                                  trh[:])
                        nc.sync.dma_start(hs3[:, t, :], hrow[:])
                        hT_prev = hT

            # ========== phase 3: ragged attention + classifier head ==========
            with tc.tile_pool(name="at", bufs=1) as at, \
                 tc.tile_pool(name="ab", bufs=4) as ab, \
                 tc.tile_pool(name="ps3", bufs=2, space="PSUM") as ps3, \
                 tc.tile_pool(name="ps4", bufs=1, space="PSUM") as ps4:
                # M: per sample the reshape-view [H, len_b] padded to T cols
                # (moff precomputed on device in phase 0)
                Mt = [at.tile([128, BC * T], BF16, tag=f"Mt{u}", name=f"Mt{u}")
                      for u in range(2)]
                # b-major issue order: with samples sorted shortest-first,
                # gather b fires as soon as the stores for steps <= lens[b]
                # land, overlapping the remaining recurrence.
                for b in range(BC):
                    for u in range(2):
                        nc.gpsimd.indirect_dma_start(
                            out=Mt[u][:, b * T:(b + 1) * T], out_offset=None,
                            in_=hs_d[:],
                            in_offset=IndirectOffsetOnAxis(
                                ap=moff[:, u * BC + b:u * BC + b + 1], axis=0))

                # q = h[len-1] per sample -> qT [128, BC] x2 (bf16). Issued AFTER
                # the M gathers: q depends on the longest sample's last
                # store, and the gpsimd queue is in-order - putting it
                # first would head-of-line block all M gathers.
                qrow = at.tile([BC, H], BF16, tag="qrow")
                nc.gpsimd.indirect_dma_start(
                    out=qrow[:], out_offset=None, in_=hs_d[:],
                    in_offset=IndirectOffsetOnAxis(ap=qoff[:], axis=0))
                qT = at.tile([128, 2 * BC], BF16, tag="qT")
                for u in range(2):
                    trq = ps3.tile([128, BC], BF16, tag="tr")
                    nc.tensor.transpose(out=trq[:],
                                        in_=qrow[:, u * 128:(u + 1) * 128],
                                        identity=idb[:BC, :BC])
                    nc.vector.tensor_copy(qT[:, u * BC:(u + 1) * BC], trq[:])

                # scores: per sample q_b . M_b -> [1, T] row, then rank-1
                # accumulate rows into a [BC, T] PSUM via one-hot columns
                eye_sb = at.tile([1, BC * BC], F32, tag="eye")
                nc.sync.dma_start(eye_sb[:], eye_d[:])
                score_ps = ps4.tile([BC, T], F32, tag="scoreacc")
                for b in range(BC):
                    scp = ps3.tile([1, T], F32, tag="sc")
                    nc.tensor.matmul(out=scp[:], lhsT=qT[:, b:b + 1],
                                     rhs=Mt[0][:, b * T:(b + 1) * T],
                                     start=True, stop=False)
                    nc.tensor.matmul(out=scp[:], lhsT=qT[:, BC + b:BC + b + 1],
                                     rhs=Mt[1][:, b * T:(b + 1) * T],
                                     start=False, stop=True)
                    rsb = ab.tile([1, T], F32, tag="rsb")
                    nc.scalar.copy(rsb[:], scp[:])
                    nc.tensor.matmul(out=score_ps[:],
                                     lhsT=eye_sb[0:1, b * BC:(b + 1) * BC],
                                     rhs=rsb[:], start=(b == 0), stop=(b == BC - 1))
                score = at.tile([BC, T], F32, tag="score")
                nc.vector.tensor_copy(score[:], score_ps[:])
                nc.vector.tensor_tensor(out=score[:], in0=score[:], in1=mask[:],
                                        op=ALU.add)
                # softmax over T (free dim)
                mx = at.tile([BC, 1], F32, tag="mx")
                nc.vector.tensor_reduce(mx[:], score[:], axis=AX.X, op=ALU.max,
                                        negate=True)
                prob = at.tile([BC, PW], F32, tag="prob")
                nc.gpsimd.memset(prob[:], 0.0)
                sm = at.tile([BC, 1], F32, tag="sm")
                nc.scalar.activation(prob[:, :T], score[:], AF.Exp,
                                     bias=mx[:, 0:1], accum_out=sm[:, 0:1])
                rs = at.tile([BC, 1], F32, tag="rs")
                nc.vector.reciprocal(rs[:], sm[:])
                nc.vector.tensor_scalar_mul(prob[:, :T], prob[:, :T], rs[:, 0:1])
                # prob^T in bf16, [128, NK*BC]
                pT = at.tile([128, NK * BC], BF16, tag="pT")
                for k in range(NK):
                    trp2 = ps3.tile([128, BC], F32, tag="tr")
                    nc.tensor.transpose(out=trp2[:],
                                        in_=prob[:, k * 128:(k + 1) * 128],
                                        identity=idf[:BC, :BC])
                    nc.vector.tensor_copy(pT[:, k * BC:(k + 1) * BC], trp2[:])

                # ctx^T [H, BC]: per sample sum_l prob[l] * hs_b[l, :]
                ctxp = [ps4.tile([128, BC], F32, tag=f"ctx{u}", name=f"ctx{u}")
                        for u in range(2)]
                for b in range(BC):
                    ob = ab.tile([128, NK * H], BF16, tag="ob")
                    for k, lk in enumerate(LCH):
                        nc.sync.dma_start(ob[:lk, k * H:k * H + H],
                                          hs3[b, k * 128:k * 128 + lk, :])
                    for u in range(2):
                        for k, lk in enumerate(LCH):
                            nc.tensor.matmul(
                                out=ctxp[u][:, b:b + 1],
                                lhsT=ob[:lk, k * H + u * 128:k * H + (u + 1) * 128],
                                rhs=pT[:lk, k * BC + b:k * BC + b + 1],
                                start=(k == 0), stop=(k == NK - 1),
                                skip_group_check=True)
                ctxT = at.tile([128, 2 * BC], BF16, tag="ctxT")
                for u in range(2):
                    nc.vector.tensor_copy(ctxT[:, u * BC:(u + 1) * BC], ctxp[u][:])

                # a^T = tanh(W1 @ [ctx; q] + b1)  [H, BC]
                rhs4 = [ctxT[:, :BC], ctxT[:, BC:], qT[:, :BC], qT[:, BC:]]
                aT = at.tile([128, 2 * BC], BF16, tag="aT")
                for m in range(2):
                    atp = ps4.tile([128, BC], F32, tag="atp")
                    for k in range(4):
                        nc.tensor.matmul(
                            out=atp[:],
                            lhsT=w1_sb[:, k * H + m * 128:k * H + (m + 1) * 128],
                            rhs=rhs4[k], start=(k == 0), stop=(k == 3))
                    nc.scalar.activation(aT[:, m * BC:(m + 1) * BC], atp[:], AF.Tanh,
                                         bias=b1_sb[:, m:m + 1])
                # logits^T [C, BC] + b2; transpose; softmax over C
                lgp = ps3.tile([C, BC], F32, tag="tr")
                nc.tensor.matmul(out=lgp[:], lhsT=w2_sb[:, :C], rhs=aT[:, :BC],
                                 start=True, stop=False)
                nc.tensor.matmul(out=lgp[:], lhsT=w2_sb[:, C:], rhs=aT[:, BC:],
                                 start=False, stop=True)
                lg = at.tile([C, BC], F32, tag="lg")
                nc.scalar.activation(lg[:], lgp[:], AF.Identity, bias=b2_sb[:, 0:1])
                lgTp = ps3.tile([BC, C], F32, tag="tr")
                nc.tensor.transpose(out=lgTp[:], in_=lg[:], identity=idf[:C, :C])
                lgT = at.tile([BC, C], F32, tag="lgT")
                nc.vector.tensor_copy(lgT[:], lgTp[:])
                mx2 = at.tile([BC, 1], F32, tag="mx2")
                nc.vector.tensor_reduce(mx2[:], lgT[:], axis=AX.X, op=ALU.max,
                                        negate=True)
                sm2 = at.tile([BC, 1], F32, tag="sm2")
                pr2 = at.tile([BC, C], F32, tag="pr2")
                nc.scalar.activation(pr2[:], lgT[:], AF.Exp, bias=mx2[:, 0:1],
                                     accum_out=sm2[:, 0:1])
                rs2 = at.tile([BC, 1], F32, tag="rs2")
                nc.vector.reciprocal(rs2[:], sm2[:])
                nc.vector.tensor_scalar_mul(pr2[:], pr2[:], rs2[:, 0:1])
                nc.sync.dma_start(out_d[:], pr2[:])
    nc.compile()
    return nc


def static_prep(emb, W_ih, W_hh, b_ih, b_hh, W1, b1, W2, b2, BC):
    """Weight/table tensors — identical for every core and (in practice)
    every call. Uploaded to device once and cached."""
    bf = ml_dtypes.bfloat16
    Hh, Cc, Gg = H, C, G
    emb_bf = np.ascontiguousarray(np.asarray(emb, np.float32).astype(bf))
    Wcat = np.concatenate([np.asarray(W_ih, np.float32),
                           np.asarray(W_hh, np.float32)], axis=1)  # [G, D+H]
    wt = np.ascontiguousarray(Wcat.T.astype(bf)).reshape(3, 128, Gg)
    w1t = np.ascontiguousarray(np.asarray(W1, np.float32).T.astype(bf)).reshape(4, 128, Hh)
    w2t = np.ascontiguousarray(np.asarray(W2, np.float32).T.astype(bf)).reshape(2, 128, Cc)
    biasg = np.ascontiguousarray(
        (np.asarray(b_ih, np.float32) + np.asarray(b_hh, np.float32))
        .reshape(8, 128).T.astype(np.float32))
    b1t = np.ascontiguousarray(np.asarray(b1, np.float32).reshape(2, 128).T)
    b2c = np.ascontiguousarray(np.asarray(b2, np.float32).reshape(Cc, 1))
    eye = np.ascontiguousarray(np.eye(BC, dtype=np.float32).reshape(1, BC * BC))
    T = L
    iotal = np.broadcast_to(np.arange(T, dtype=np.float32), (BC, T)).copy()
    bth = np.broadcast_to((np.arange(BC) * T * Hh).astype(np.float32), (128, BC)).copy()
    iotap = np.stack([np.arange(128, dtype=np.float32),
                      np.arange(128, dtype=np.float32) + 128.0], axis=1)
    qbias = (np.arange(BC, dtype=np.float32) * T * Hh - Hh).reshape(BC, 1)
    return dict(emb=emb_bf, wt=wt, w1t=w1t, w2t=w2t, biasg=biasg, b1t=b1t,
                b2c=b2c, eye=eye, iotal=iotal, bth=bth,
                iotap=np.ascontiguousarray(iotap), qbias=qbias)


def dynamic_prep(inputs_arrays, traj_lens, BC, T):
    """Per-call tensor: token tiles + sorted lens packed into one i32 buffer
    (~77KB/core)."""
    n_cores = np.asarray(inputs_arrays).shape[0] // BC
    idx_all = np.asarray(inputs_arrays).astype(np.int64)
    lens_all = np.asarray(traj_lens).astype(np.int64)
    NT = BC * T // 128
    per_core = []
    orders = []
    for c in range(n_cores):
        idx = idx_all[c * BC:(c + 1) * BC]          # [BC, T]
        lens = lens_all[c * BC:(c + 1) * BC]        # [BC]
        # Sort samples shortest-first within the core: M-gather slot b then
        # depends only on recurrence steps <= lens[b], so the in-order gpsimd
        # queue drains progressively during the recurrence instead of
        # serializing after it. Rows are un-permuted host-side in kernel().
        order = np.argsort(lens, kind="stable")
        orders.append(order)
        idx = idx[order]
        lens = lens[order]
        tok = idx.T.reshape(-1)                     # t-major token order
        dynbuf = np.zeros((128, NT + 1), np.int32)
        dynbuf[:, :NT] = tok.reshape(NT, 128).astype(np.int32).T
        dynbuf[:BC, NT] = lens.astype(np.int32)
        per_core.append(dict(dynbuf=dynbuf))
    return per_core, orders


_CACHE = {}


def _weights_digest(inputs):
    """Cheap content fingerprint of the weight tensors so cached device
    copies are refreshed if a caller ever changes them between calls."""
    parts = []
    for k in ("emb", "W_ih", "W_hh", "b_ih", "b_hh", "W1", "b1", "W2", "b2"):
        a = np.ascontiguousarray(np.asarray(inputs[k]))
        if a.nbytes % 8 == 0:
            v = a.reshape(-1).view(np.uint64)
        elif a.nbytes % 4 == 0:
            v = a.reshape(-1).view(np.uint32)
        else:
            v = a.reshape(-1).view(np.uint8)
        parts.append((a.shape, str(a.dtype), int(np.sum(v, dtype=np.uint64))))
    return tuple(parts)


def _get_runtime():
    """Build the Bass program once and wrap it in a cached jitted shard_map
    (mirrors concourse.bass2jax.run_bass_via_pjrt, but reusable across calls
    so compiled executable + device-resident weights are not re-shipped)."""
    if "rt" in _CACHE:
        return _CACHE["rt"]
    import jax
    from jax.experimental.shard_map import shard_map
    from jax.sharding import Mesh, NamedSharding, PartitionSpec
    from concourse.bass2jax import (_bass_exec_p, install_neuronx_cc_hook,
                                    partition_id_tensor)

    install_neuronx_cc_hook()
    nc = build_kernel(BC=B // NCORES, T=L, VV=V, CH_STEPS=20)
    partition_name = (nc.partition_id_tensor.name
                      if nc.partition_id_tensor else None)
    in_names, out_names, out_avals, zero_outs = [], [], [], []
    for alloc in nc.m.functions[0].allocations:
        if not isinstance(alloc, mybir.MemoryLocationSet):
            continue
        name = alloc.memorylocations[0].name
        if alloc.kind == "ExternalInput":
            if name != partition_name:
                in_names.append(name)
        elif alloc.kind == "ExternalOutput":
            out_names.append(name)
            shape = tuple(alloc.tensor_shape)
            dtype = mybir.dt.np(alloc.dtype)
            out_avals.append(jax.core.ShapedArray(shape, dtype))
            zero_outs.append(np.zeros((NCORES * shape[0], *shape[1:]), dtype))
    n_params = len(in_names)
    n_outs = len(out_names)
    in_names_all = list(in_names) + list(out_names)
    if partition_name is not None:
        in_names_all.append(partition_name)
    donate = tuple(range(n_params, n_params + n_outs))

    def _body(*args):
        operands = list(args)
        if partition_name is not None:
            operands.append(partition_id_tensor())
        outs = _bass_exec_p.bind(
            *operands, out_avals=tuple(out_avals), in_names=tuple(in_names_all),
            out_names=tuple(out_names), lowering_input_output_aliases=(),
            sim_require_finite=True, sim_require_nnan=True, nc=nc)
        return tuple(outs)

    devices = jax.devices()[:NCORES]
    assert len(devices) == NCORES
    mesh = Mesh(np.asarray(devices), ("core",))
    in_specs = (PartitionSpec("core"),) * (n_params + n_outs)
    out_specs = (PartitionSpec("core"),) * n_outs
    sharded = jax.jit(
        shard_map(_body, mesh=mesh, in_specs=in_specs, out_specs=out_specs,
                  check_rep=False),
        donate_argnums=donate, keep_unused=True)
    rt = dict(sharded=sharded, in_names=in_names, out_names=out_names,
              zero_outs=zero_outs,
              sharding=NamedSharding(mesh, PartitionSpec("core")))
    _CACHE["rt"] = rt
    return rt


def _upload_static(inputs, rt, BC):
    import jax
    st = static_prep(inputs["emb"], inputs["W_ih"], inputs["W_hh"],
                     inputs["b_ih"], inputs["b_hh"], inputs["W1"],
                     inputs["b1"], inputs["W2"], inputs["b2"], BC=BC)
    _CACHE["static_dev"] = {
        k: jax.device_put(np.concatenate([v] * NCORES, axis=0),
                          rt["sharding"])
        for k, v in st.items()}


def _dispatch(rt, dyn):
    args = [_CACHE["static_dev"].get(nm, dyn.get(nm))
            for nm in rt["in_names"]] + list(rt["zero_outs"])
    return rt["sharded"](*args)


def kernel(**inputs):
    BC = B // NCORES
    rt = _get_runtime()

    per_core, orders = dynamic_prep(inputs["inputs_arrays"],
                                    inputs["traj_lens"], BC=BC, T=L)
    dyn = {k: np.concatenate([m[k] for m in per_core], axis=0)
           for k in per_core[0]}

    if "static_dev" in _CACHE:
        # optimistic: dispatch with the cached weight tensors, verify the
        # fingerprint while the device runs, re-run only on a (rare) change
        out_arrs = _dispatch(rt, dyn)
        digest = _weights_digest(inputs)
        if _CACHE.get("static_digest") != digest:
            _upload_static(inputs, rt, BC)
            _CACHE["static_digest"] = digest
            out_arrs = _dispatch(rt, dyn)
    else:
        digest = _weights_digest(inputs)
        _upload_static(inputs, rt, BC)
        _CACHE["static_digest"] = digest
        out_arrs = _dispatch(rt, dyn)

    res = np.asarray(out_arrs[rt["out_names"].index("out")], np.float32)
    res = res.reshape(NCORES, BC, C)
    out = np.empty((B, C), np.float32)
    for c in range(NCORES):
        out[c * BC + orders[c]] = res[c]
    return out



# revision 15
# speedup vs baseline: 1.2702x; 1.2702x over previous
"""Trainium2 Bass kernel for nn_ClassificationRNN2 (embedding + LSTM + ragged attention + head).

Strategy: data-parallel over batch across 8 NeuronCores (64 samples/core),
weights/embedding replicated, no collectives.

Host/runtime: the axon tunnel runs at ~35MB/s, so the warm-call cost is all
input transfer. The jitted shard_map executable is cached across calls and
the weight/embedding tensors live device-resident (refreshed only when a
content fingerprint changes, verified concurrently with the dispatch). Each
call ships one small i32 buffer per core (token tiles + sorted lens, ~77KB);
all ragged-length-dependent addressing (q/M gather offsets, softmax mask) is
derived from lens on device in exact f32.

Per-core layout: "transposed" H-major state. Per step t:
  g^T[1024,64] = Wcat^T.T @ [x_t; h_{t-1}]  (24 bf16 matmuls, fp32 PSUM)
  gates on ACT (sigmoid/tanh share one table set), cell update on DVE,
  h_t transposed (PE) to b-major and stored to a DRAM scratch [BC,T,H].
Attention reads that scratch: q via indirect gather at len-1, the ragged
reshape-view M[b] = flat_b.reshape(H, len_b) via indirect gather with
offsets h*len_b, score/ctx via per-sample matmuls.
"""

import numpy as np
import ml_dtypes

import concourse.bass as bass
import concourse.mybir as mybir
import concourse.tile as tile
from concourse import bacc
from concourse.bass import IndirectOffsetOnAxis
from concourse.masks import make_identity

BF16 = mybir.dt.bfloat16
F32 = mybir.dt.float32
I32 = mybir.dt.int32
AF = mybir.ActivationFunctionType
ALU = mybir.AluOpType
AX = mybir.AxisListType

NCORES = 8
B, L, D, H, V, C = 512, 300, 128, 256, 100001, 14
G = 4 * H  # 1024 gate dims


def build_kernel(BC, T, VV, CH_STEPS, enable_asserts=False):
    """Per-core program. BC=batch/core, T=steps, VV=vocab rows,
    CH_STEPS*BC must be a multiple of 128 and divide BC*T."""
    TOK = BC * T
    TOK_CH = BC * CH_STEPS
    assert TOK_CH % 128 == 0 and TOK % TOK_CH == 0
    TPC = TOK_CH // 128          # 128-token transpose tiles per chunk
    NCH = TOK // TOK_CH          # x^T chunks
    PW = ((T + 127) // 128) * 128
    NK = PW // 128               # l-chunks for ctx
    LCH = [min(128, T - k * 128) for k in range(NK)]

    nc = bacc.Bacc("TRN2", target_bir_lowering=False, debug=False,
                   enable_asserts=enable_asserts)

    # ---- DRAM I/O ----
    emb_d = nc.dram_tensor("emb", [VV, D], BF16, kind="ExternalInput")
    wt_d = nc.dram_tensor("wt", [3, 128, G], BF16, kind="ExternalInput")
    w1t_d = nc.dram_tensor("w1t", [4, 128, H], BF16, kind="ExternalInput")
    w2t_d = nc.dram_tensor("w2t", [2, 128, C], BF16, kind="ExternalInput")
    biasg_d = nc.dram_tensor("biasg", [128, 8], F32, kind="ExternalInput")
    b1t_d = nc.dram_tensor("b1t", [128, 2], F32, kind="ExternalInput")
    b2c_d = nc.dram_tensor("b2c", [C, 1], F32, kind="ExternalInput")
    NT = TOK // 128              # total 128-token tiles
    # single per-call tensor: cols 0..NT-1 token tiles, col NT = sorted lens
    dynbuf_d = nc.dram_tensor("dynbuf", [128, NT + 1], I32, kind="ExternalInput")
    # static iota/address-helper constants (uploaded once with the weights)
    iotal_d = nc.dram_tensor("iotal", [BC, T], F32, kind="ExternalInput")
    bth_d = nc.dram_tensor("bth", [128, BC], F32, kind="ExternalInput")
    iotap_d = nc.dram_tensor("iotap", [128, 2], F32, kind="ExternalInput")
    qbias_d = nc.dram_tensor("qbias", [BC, 1], F32, kind="ExternalInput")
    eye_d = nc.dram_tensor("eye", [1, BC * BC], F32, kind="ExternalInput")
    out_d = nc.dram_tensor("out", [BC, C], F32, kind="ExternalOutput")
    # internal DRAM scratch: per-sample row-major hidden states, flat for gathers
    hs_d = nc.dram_tensor("hsflat", [BC * T * H, 1], BF16)
    hs3 = hs_d[:].rearrange("(b t h) one -> b t (h one)", b=BC, t=T)

    with tile.TileContext(nc) as tc:
        with tc.tile_pool(name="persist", bufs=1) as pp:
            # ---- persistent SBUF ----
            idf = pp.tile([128, 128], F32, tag="idf")
            make_identity(nc, idf[:])
            idb = pp.tile([128, 128], BF16, tag="idb")
            nc.vector.tensor_copy(idb[:], idf[:])

            w_sb = pp.tile([128, 3 * G], BF16, tag="w")
            w1_sb = pp.tile([128, 4 * H], BF16, tag="w1")
            w2_sb = pp.tile([128, 2 * C], BF16, tag="w2")
            for k in range(3):
                nc.sync.dma_start(w_sb[:, k * G:(k + 1) * G], wt_d[k])
            for k in range(4):
                nc.sync.dma_start(w1_sb[:, k * H:(k + 1) * H], w1t_d[k])
            for k in range(2):
                nc.sync.dma_start(w2_sb[:, k * C:(k + 1) * C], w2t_d[k])
            bg_sb = pp.tile([128, 8], F32, tag="bg")
            nc.sync.dma_start(bg_sb[:], biasg_d[:])
            b1_sb = pp.tile([128, 2], F32, tag="b1")
            nc.sync.dma_start(b1_sb[:], b1t_d[:])
            b2_sb = pp.tile([C, 1], F32, tag="b2")
            nc.sync.dma_start(b2_sb[:], b2c_d[:])

            xT = [pp.tile([128, TOK_CH], BF16, tag=f"xT{c}", name=f"xT{c}")
                  for c in range(NCH)]

            # ========== phase 0: ragged address math from lens ==========
            # dynbuf carries tokens + sorted lens; derive qoff/moff/mask here
            # (f32 is exact: all values < 2^24).
            idx_all = pp.tile([128, NT + 1], I32, tag="idx")
            nc.sync.dma_start(idx_all[:], dynbuf_d[:])
            iotal_sb = pp.tile([BC, T], F32, tag="iotal")
            nc.sync.dma_start(iotal_sb[:], iotal_d[:])
            bth_sb = pp.tile([128, BC], F32, tag="bth")
            nc.sync.dma_start(bth_sb[:], bth_d[:])
            iotap_sb = pp.tile([128, 2], F32, tag="iotap")
            nc.sync.dma_start(iotap_sb[:], iotap_d[:])
            qbias_sb = pp.tile([BC, 1], F32, tag="qbias")
            nc.sync.dma_start(qbias_sb[:], qbias_d[:])
            ones_sb = pp.tile([1, 128], F32, tag="ones")
            nc.vector.memset(ones_sb[:], 1.0)

            lens_f = pp.tile([BC, 1], F32, tag="lensf")
            nc.vector.tensor_copy(lens_f[:], idx_all[:BC, NT:NT + 1])
            moff = pp.tile([128, 2 * BC], I32, tag="moff")
            qoff = pp.tile([BC, 1], I32, tag="qoff")
            mask = pp.tile([BC, T], F32, tag="mask")
            qf = pp.tile([BC, 1], F32, tag="qf")
            # qoff = lens*H + (b*T*H - H)
            nc.vector.tensor_scalar(qf[:], lens_f[:], float(H),
                                    scalar2=qbias_sb[:, 0:1],
                                    op0=ALU.mult, op1=ALU.add)
            nc.vector.tensor_copy(qoff[:], qf[:])
            # mask = (l >= lens) * -1e30
            nc.vector.tensor_scalar(mask[:], iotal_sb[:], lens_f[:, 0:1],
                                    scalar2=-1e30,
                                    op0=ALU.is_ge, op1=ALU.mult)
            with tc.tile_pool(name="p0", bufs=1, space="PSUM") as ps0:
                # lens broadcast over partitions: lensB[p,b] = lens[b]
                lrp = ps0.tile([1, BC], F32, tag="lrp")
                nc.tensor.matmul(out=lrp[:], lhsT=lens_f[:], rhs=idf[:BC, :BC],
                                 start=True, stop=True)
                lens_row = pp.tile([1, BC], F32, tag="lensrow")
                nc.vector.tensor_copy(lens_row[:], lrp[:])
                lBp = ps0.tile([128, BC], F32, tag="lBp")
                nc.tensor.matmul(out=lBp[:], lhsT=ones_sb[:], rhs=lens_row[:],
                                 start=True, stop=True)
                lensB = pp.tile([128, BC], F32, tag="lensB")
                nc.vector.tensor_copy(lensB[:], lBp[:])
            # moff[u][p,b] = (u*128+p)*lens[b] + b*T*H
            mf = pp.tile([128, 2 * BC], F32, tag="mf")
            for u in range(2):
                cs = slice(u * BC, (u + 1) * BC)
                nc.vector.tensor_scalar(mf[:, cs], lensB[:], iotap_sb[:, u:u + 1],
                                        scalar2=None, op0=ALU.mult)
                nc.vector.tensor_tensor(out=mf[:, cs], in0=mf[:, cs],
                                        in1=bth_sb[:], op=ALU.add)
            nc.vector.tensor_copy(moff[:], mf[:])

            # ========== phase 1: embedding gather + transpose to x^T ==========
            # HW indirect DMA consumes ONE offset per dest partition; the whole
            # per-partition free extent streams contiguously from it. So gather
            # one 128-token tile ([128, D]) per instruction.
            with tc.tile_pool(name="gat", bufs=4) as gp, \
                 tc.tile_pool(name="ps1", bufs=2, space="PSUM") as ps1:
                for g in range(NT):
                    ci, j = g // TPC, g % TPC
                    xrows = gp.tile([128, 128], BF16, tag="xrows")
                    nc.gpsimd.indirect_dma_start(
                        out=xrows[:], out_offset=None,
                        in_=emb_d[:],
                        in_offset=IndirectOffsetOnAxis(ap=idx_all[:, g:g + 1],
                                                       axis=0),
                    )
                    trp = ps1.tile([128, 128], BF16, tag="trx")
                    nc.tensor.transpose(out=trp[:], in_=xrows[:],
                                        identity=idb[:])
                    nc.vector.tensor_copy(
                        xT[ci][:, j * 128:(j + 1) * 128], trp[:])

                # ========== phase 2: LSTM recurrence ==========
                with tc.tile_pool(name="st", bufs=1) as sp, \
                     tc.tile_pool(name="lp", bufs=2) as lp, \
                     tc.tile_pool(name="ps2", bufs=2, space="PSUM") as ps2:
                    c_sb = sp.tile([128, 2 * BC], F32, tag="c")
                    nc.gpsimd.memset(c_sb[:], 0.0)
                    hT_prev = lp.tile([128, 2 * BC], BF16, tag="hT")
                    nc.gpsimd.memset(hT_prev[:], 0.0)

                    for t in range(T):
                        ch, col = t // CH_STEPS, (t % CH_STEPS) * BC
                        xcol = xT[ch][:, col:col + BC]
                        gA = ps2.tile([128, 4 * BC], F32, tag="gA")
                        gB = ps2.tile([128, 4 * BC], F32, tag="gB")
                        for j in range(8):
                            out = (gA if j < 4 else gB)[:, (j % 4) * BC:(j % 4 + 1) * BC]
                            wj = slice(j * 128, (j + 1) * 128)
                            nc.tensor.matmul(out=out, lhsT=w_sb[:, wj], rhs=xcol,
                                             start=True, stop=False)
                            nc.tensor.matmul(out=out, lhsT=w_sb[:, G:][:, wj],
                                             rhs=hT_prev[:, :BC], start=False, stop=False)
                            nc.tensor.matmul(out=out, lhsT=w_sb[:, 2 * G:][:, wj],
                                             rhs=hT_prev[:, BC:], start=False, stop=True)
                        # gates: i=j0,1  f=j2,3 (gA)   g~=j4,5  o=j6,7 (gB)
                        i_sb = lp.tile([128, 2 * BC], F32, tag="i")
                        f_sb = lp.tile([128, 2 * BC], F32, tag="f")
                        g_sb = lp.tile([128, 2 * BC], F32, tag="g")
                        o_sb = lp.tile([128, 2 * BC], F32, tag="o")
                        for u in range(2):
                            cs = slice(u * BC, (u + 1) * BC)
                            cs2 = slice(2 * BC + u * BC, 2 * BC + (u + 1) * BC)
                            nc.scalar.activation(i_sb[:, cs], gA[:, cs], AF.Sigmoid,
                                                 bias=bg_sb[:, u:u + 1])
                            nc.scalar.activation(f_sb[:, cs], gA[:, cs2], AF.Sigmoid,
                                                 bias=bg_sb[:, 2 + u:3 + u])
                            nc.scalar.activation(g_sb[:, cs], gB[:, cs], AF.Tanh,
                                                 bias=bg_sb[:, 4 + u:5 + u])
                            nc.scalar.activation(o_sb[:, cs], gB[:, cs2], AF.Sigmoid,
                                                 bias=bg_sb[:, 6 + u:7 + u])
                        t1 = lp.tile([128, 2 * BC], F32, tag="t1")
                        nc.vector.tensor_tensor(out=t1[:], in0=i_sb[:], in1=g_sb[:],
                                                op=ALU.mult)
                        nc.vector.tensor_tensor(out=c_sb[:], in0=c_sb[:], in1=f_sb[:],
                                                op=ALU.mult)
                        nc.vector.tensor_tensor(out=c_sb[:], in0=c_sb[:], in1=t1[:],
                                                op=ALU.add)
                        th = lp.tile([128, 2 * BC], F32, tag="th")
                        nc.scalar.activation(th[:], c_sb[:], AF.Tanh)
                        hT = lp.tile([128, 2 * BC], BF16, tag="hT")
                        nc.vector.tensor_tensor(out=hT[:], in0=o_sb[:], in1=th[:],
                                                op=ALU.mult)
                        # b-major row to DRAM for the attention phase
                        hrow = lp.tile([BC, H], BF16, tag="hrow")
                        for u in range(2):
                            trh = ps2.tile([BC, 128], BF16, tag="trh")
                            nc.tensor.transpose(out=trh[:],
                                                in_=hT[:, u * BC:(u + 1) * BC],
                                                identity=idb[:])
                            nc.vector.tensor_copy(hrow[:, u * 128:(u + 1) * 128],
                                                  trh[:])
                        nc.sync.dma_start(hs3[:, t, :], hrow[:])
                        hT_prev = hT

            # ========== phase 3: ragged attention + classifier head ==========
            with tc.tile_pool(name="at", bufs=1) as at, \
                 tc.tile_pool(name="ab", bufs=4) as ab, \
                 tc.tile_pool(name="ps3", bufs=2, space="PSUM") as ps3, \
                 tc.tile_pool(name="ps4", bufs=1, space="PSUM") as ps4:
                # M: per sample the reshape-view [H, len_b] padded to T cols
                # (moff precomputed on device in phase 0)
                Mt = [at.tile([128, BC * T], BF16, tag=f"Mt{u}", name=f"Mt{u}")
                      for u in range(2)]
                # b-major issue order: with samples sorted shortest-first,
                # gather b fires as soon as the stores for steps <= lens[b]
                # land, overlapping the remaining recurrence.
                for b in range(BC):
                    for u in range(2):
                        nc.gpsimd.indirect_dma_start(
                            out=Mt[u][:, b * T:(b + 1) * T], out_offset=None,
                            in_=hs_d[:],
                            in_offset=IndirectOffsetOnAxis(
                                ap=moff[:, u * BC + b:u * BC + b + 1], axis=0))

                # q = h[len-1] per sample -> qT [128, BC] x2 (bf16). Issued AFTER
                # the M gathers: q depends on the longest sample's last
                # store, and the gpsimd queue is in-order - putting it
                # first would head-of-line block all M gathers.
                qrow = at.tile([BC, H], BF16, tag="qrow")
                nc.gpsimd.indirect_dma_start(
                    out=qrow[:], out_offset=None, in_=hs_d[:],
                    in_offset=IndirectOffsetOnAxis(ap=qoff[:], axis=0))
                qT = at.tile([128, 2 * BC], BF16, tag="qT")
                for u in range(2):
                    trq = ps3.tile([128, BC], BF16, tag="tr")
                    nc.tensor.transpose(out=trq[:],
                                        in_=qrow[:, u * 128:(u + 1) * 128],
                                        identity=idb[:BC, :BC])
                    nc.vector.tensor_copy(qT[:, u * BC:(u + 1) * BC], trq[:])

                # scores: per sample q_b . M_b -> [1, T] row, then rank-1
                # accumulate rows into a [BC, T] PSUM via one-hot columns
                eye_sb = at.tile([1, BC * BC], F32, tag="eye")
                nc.sync.dma_start(eye_sb[:], eye_d[:])
                score_ps = ps4.tile([BC, T], F32, tag="scoreacc")
                for b in range(BC):
                    scp = ps3.tile([1, T], F32, tag="sc")
                    nc.tensor.matmul(out=scp[:], lhsT=qT[:, b:b + 1],
                                     rhs=Mt[0][:, b * T:(b + 1) * T],
                                     start=True, stop=False)
                    nc.tensor.matmul(out=scp[:], lhsT=qT[:, BC + b:BC + b + 1],
                                     rhs=Mt[1][:, b * T:(b + 1) * T],
                                     start=False, stop=True)
                    rsb = ab.tile([1, T], F32, tag="rsb")
                    nc.scalar.copy(rsb[:], scp[:])
                    nc.tensor.matmul(out=score_ps[:],
                                     lhsT=eye_sb[0:1, b * BC:(b + 1) * BC],
                                     rhs=rsb[:], start=(b == 0), stop=(b == BC - 1))
                score = at.tile([BC, T], F32, tag="score")
                nc.vector.tensor_copy(score[:], score_ps[:])
                nc.vector.tensor_tensor(out=score[:], in0=score[:], in1=mask[:],
                                        op=ALU.add)
                # softmax over T (free dim)
                mx = at.tile([BC, 1], F32, tag="mx")
                nc.vector.tensor_reduce(mx[:], score[:], axis=AX.X, op=ALU.max,
                                        negate=True)
                prob = at.tile([BC, PW], F32, tag="prob")
                nc.gpsimd.memset(prob[:], 0.0)
                sm = at.tile([BC, 1], F32, tag="sm")
                nc.scalar.activation(prob[:, :T], score[:], AF.Exp,
                                     bias=mx[:, 0:1], accum_out=sm[:, 0:1])
                rs = at.tile([BC, 1], F32, tag="rs")
                nc.vector.reciprocal(rs[:], sm[:])
                nc.vector.tensor_scalar_mul(prob[:, :T], prob[:, :T], rs[:, 0:1])
                # prob^T in bf16, [128, NK*BC]
                pT = at.tile([128, NK * BC], BF16, tag="pT")
                for k in range(NK):
                    trp2 = ps3.tile([128, BC], F32, tag="tr")
                    nc.tensor.transpose(out=trp2[:],
                                        in_=prob[:, k * 128:(k + 1) * 128],
                                        identity=idf[:BC, :BC])
                    nc.vector.tensor_copy(pT[:, k * BC:(k + 1) * BC], trp2[:])

                # ctx^T [H, BC]: per sample sum_l prob[l] * hs_b[l, :]
                ctxp = [ps4.tile([128, BC], F32, tag=f"ctx{u}", name=f"ctx{u}")
                        for u in range(2)]
                for b in range(BC):
                    ob = ab.tile([128, NK * H], BF16, tag="ob")
                    for k, lk in enumerate(LCH):
                        nc.sync.dma_start(ob[:lk, k * H:k * H + H],
                                          hs3[b, k * 128:k * 128 + lk, :])
                    for u in range(2):
                        for k, lk in enumerate(LCH):
                            nc.tensor.matmul(
                                out=ctxp[u][:, b:b + 1],
                                lhsT=ob[:lk, k * H + u * 128:k * H + (u + 1) * 128],
                                rhs=pT[:lk, k * BC + b:k * BC + b + 1],
                                start=(k == 0), stop=(k == NK - 1),
                                skip_group_check=True)
                ctxT = at.tile([128, 2 * BC], BF16, tag="ctxT")
                for u in range(2):
                    nc.vector.tensor_copy(ctxT[:, u * BC:(u + 1) * BC], ctxp[u][:])

                # a^T = tanh(W1 @ [ctx; q] + b1)  [H, BC]
                rhs4 = [ctxT[:, :BC], ctxT[:, BC:], qT[:, :BC], qT[:, BC:]]
                aT = at.tile([128, 2 * BC], BF16, tag="aT")
                for m in range(2):
                    atp = ps4.tile([128, BC], F32, tag="atp")
                    for k in range(4):
                        nc.tensor.matmul(
                            out=atp[:],
                            lhsT=w1_sb[:, k * H + m * 128:k * H + (m + 1) * 128],
                            rhs=rhs4[k], start=(k == 0), stop=(k == 3))
                    nc.scalar.activation(aT[:, m * BC:(m + 1) * BC], atp[:], AF.Tanh,
                                         bias=b1_sb[:, m:m + 1])
                # logits^T [C, BC] + b2; transpose; softmax over C
                lgp = ps3.tile([C, BC], F32, tag="tr")
                nc.tensor.matmul(out=lgp[:], lhsT=w2_sb[:, :C], rhs=aT[:, :BC],
                                 start=True, stop=False)
                nc.tensor.matmul(out=lgp[:], lhsT=w2_sb[:, C:], rhs=aT[:, BC:],
                                 start=False, stop=True)
                lg = at.tile([C, BC], F32, tag="lg")
                nc.scalar.activation(lg[:], lgp[:], AF.Identity, bias=b2_sb[:, 0:1])
                lgTp = ps3.tile([BC, C], F32, tag="tr")
                nc.tensor.transpose(out=lgTp[:], in_=lg[:], identity=idf[:C, :C])
                lgT = at.tile([BC, C], F32, tag="lgT")
                nc.vector.tensor_copy(lgT[:], lgTp[:])
                mx2 = at.tile([BC, 1], F32, tag="mx2")
                nc.vector.tensor_reduce(mx2[:], lgT[:], axis=AX.X, op=ALU.max,
                                        negate=True)
                sm2 = at.tile([BC, 1], F32, tag="sm2")
                pr2 = at.tile([BC, C], F32, tag="pr2")
                nc.scalar.activation(pr2[:], lgT[:], AF.Exp, bias=mx2[:, 0:1],
                                     accum_out=sm2[:, 0:1])
                rs2 = at.tile([BC, 1], F32, tag="rs2")
                nc.vector.reciprocal(rs2[:], sm2[:])
                nc.vector.tensor_scalar_mul(pr2[:], pr2[:], rs2[:, 0:1])
                nc.sync.dma_start(out_d[:], pr2[:])
    nc.compile()
    return nc


def static_prep(emb, W_ih, W_hh, b_ih, b_hh, W1, b1, W2, b2, BC):
    """Weight/table tensors — identical for every core and (in practice)
    every call. Uploaded to device once and cached."""
    bf = ml_dtypes.bfloat16
    Hh, Cc, Gg = H, C, G
    emb_bf = np.ascontiguousarray(np.asarray(emb, np.float32).astype(bf))
    Wcat = np.concatenate([np.asarray(W_ih, np.float32),
                           np.asarray(W_hh, np.float32)], axis=1)  # [G, D+H]
    wt = np.ascontiguousarray(Wcat.T.astype(bf)).reshape(3, 128, Gg)
    w1t = np.ascontiguousarray(np.asarray(W1, np.float32).T.astype(bf)).reshape(4, 128, Hh)
    w2t = np.ascontiguousarray(np.asarray(W2, np.float32).T.astype(bf)).reshape(2, 128, Cc)
    biasg = np.ascontiguousarray(
        (np.asarray(b_ih, np.float32) + np.asarray(b_hh, np.float32))
        .reshape(8, 128).T.astype(np.float32))
    b1t = np.ascontiguousarray(np.asarray(b1, np.float32).reshape(2, 128).T)
    b2c = np.ascontiguousarray(np.asarray(b2, np.float32).reshape(Cc, 1))
    eye = np.ascontiguousarray(np.eye(BC, dtype=np.float32).reshape(1, BC * BC))
    T = L
    iotal = np.broadcast_to(np.arange(T, dtype=np.float32), (BC, T)).copy()
    bth = np.broadcast_to((np.arange(BC) * T * Hh).astype(np.float32), (128, BC)).copy()
    iotap = np.stack([np.arange(128, dtype=np.float32),
                      np.arange(128, dtype=np.float32) + 128.0], axis=1)
    qbias = (np.arange(BC, dtype=np.float32) * T * Hh - Hh).reshape(BC, 1)
    return dict(emb=emb_bf, wt=wt, w1t=w1t, w2t=w2t, biasg=biasg, b1t=b1t,
                b2c=b2c, eye=eye, iotal=iotal, bth=bth,
                iotap=np.ascontiguousarray(iotap), qbias=qbias)


def dynamic_prep(inputs_arrays, traj_lens, BC, T):
    """Per-call tensor: token tiles + sorted lens packed into one i32 buffer
    (~77KB/core), built directly in global (8*128, NT+1) layout."""
    n_cores = np.asarray(inputs_arrays).shape[0] // BC
    idx_all = np.asarray(inputs_arrays).astype(np.int64)
    lens_all = np.asarray(traj_lens).astype(np.int64)
    NT = BC * T // 128
    dynbuf = np.zeros((n_cores * 128, NT + 1), np.int32)
    orders = []
    for c in range(n_cores):
        idx = idx_all[c * BC:(c + 1) * BC]          # [BC, T]
        lens = lens_all[c * BC:(c + 1) * BC]        # [BC]
        # Sort samples shortest-first within the core: M-gather slot b then
        # depends only on recurrence steps <= lens[b], so the in-order gpsimd
        # queue drains progressively during the recurrence instead of
        # serializing after it. Rows are un-permuted host-side in kernel().
        order = np.argsort(lens, kind="stable")
        orders.append(order)
        idx = idx[order]
        lens = lens[order]
        tok = idx.T.reshape(-1)                     # t-major token order
        blk = dynbuf[c * 128:(c + 1) * 128]
        blk[:, :NT] = tok.reshape(NT, 128).astype(np.int32).T
        blk[:BC, NT] = lens.astype(np.int32)
    return dynbuf, orders


_CACHE = {}


def _weights_digest(inputs):
    """Cheap content fingerprint of the weight tensors so cached device
    copies are refreshed if a caller ever changes them between calls."""
    parts = []
    for k in ("emb", "W_ih", "W_hh", "b_ih", "b_hh", "W1", "b1", "W2", "b2"):
        a = np.ascontiguousarray(np.asarray(inputs[k]))
        if a.nbytes % 8 == 0:
            v = a.reshape(-1).view(np.uint64)
        elif a.nbytes % 4 == 0:
            v = a.reshape(-1).view(np.uint32)
        else:
            v = a.reshape(-1).view(np.uint8)
        parts.append((a.shape, str(a.dtype), int(np.sum(v, dtype=np.uint64))))
    return tuple(parts)


def _get_runtime():
    """Build the Bass program once and wrap it in a cached jitted shard_map
    (mirrors concourse.bass2jax.run_bass_via_pjrt, but reusable across calls
    so compiled executable + device-resident weights are not re-shipped)."""
    if "rt" in _CACHE:
        return _CACHE["rt"]
    import jax
    from jax.experimental.shard_map import shard_map
    from jax.sharding import Mesh, NamedSharding, PartitionSpec
    from concourse.bass2jax import (_bass_exec_p, install_neuronx_cc_hook,
                                    partition_id_tensor)

    install_neuronx_cc_hook()
    nc = build_kernel(BC=B // NCORES, T=L, VV=V, CH_STEPS=20)
    partition_name = (nc.partition_id_tensor.name
                      if nc.partition_id_tensor else None)
    in_names, out_names, out_avals, zero_outs = [], [], [], []
    for alloc in nc.m.functions[0].allocations:
        if not isinstance(alloc, mybir.MemoryLocationSet):
            continue
        name = alloc.memorylocations[0].name
        if alloc.kind == "ExternalInput":
            if name != partition_name:
                in_names.append(name)
        elif alloc.kind == "ExternalOutput":
            out_names.append(name)
            shape = tuple(alloc.tensor_shape)
            dtype = mybir.dt.np(alloc.dtype)
            out_avals.append(jax.core.ShapedArray(shape, dtype))
            zero_outs.append(np.zeros((NCORES * shape[0], *shape[1:]), dtype))
    n_params = len(in_names)
    n_outs = len(out_names)
    in_names_all = list(in_names) + list(out_names)
    if partition_name is not None:
        in_names_all.append(partition_name)
    donate = tuple(range(n_params, n_params + n_outs))

    def _body(*args):
        operands = list(args)
        if partition_name is not None:
            operands.append(partition_id_tensor())
        outs = _bass_exec_p.bind(
            *operands, out_avals=tuple(out_avals), in_names=tuple(in_names_all),
            out_names=tuple(out_names), lowering_input_output_aliases=(),
            sim_require_finite=True, sim_require_nnan=True, nc=nc)
        return tuple(outs)

    devices = jax.devices()[:NCORES]
    assert len(devices) == NCORES
    mesh = Mesh(np.asarray(devices), ("core",))
    in_specs = (PartitionSpec("core"),) * (n_params + n_outs)
    out_specs = (PartitionSpec("core"),) * n_outs
    sharded = jax.jit(
        shard_map(_body, mesh=mesh, in_specs=in_specs, out_specs=out_specs,
                  check_rep=False),
        donate_argnums=donate, keep_unused=True)
    rt = dict(sharded=sharded, in_names=in_names, out_names=out_names,
              zero_outs=zero_outs,
              sharding=NamedSharding(mesh, PartitionSpec("core")))
    _CACHE["rt"] = rt
    return rt


def _upload_static(inputs, rt, BC):
    import jax
    st = static_prep(inputs["emb"], inputs["W_ih"], inputs["W_hh"],
                     inputs["b_ih"], inputs["b_hh"], inputs["W1"],
                     inputs["b1"], inputs["W2"], inputs["b2"], BC=BC)
    _CACHE["static_dev"] = {
        k: jax.device_put(np.concatenate([v] * NCORES, axis=0),
                          rt["sharding"])
        for k, v in st.items()}


def _dispatch(rt, dyn):
    args = [_CACHE["static_dev"].get(nm, dyn.get(nm))
            for nm in rt["in_names"]] + list(rt["zero_outs"])
    return rt["sharded"](*args)


def kernel(**inputs):
    BC = B // NCORES
    rt = _get_runtime()

    dynbuf, orders = dynamic_prep(inputs["inputs_arrays"],
                                  inputs["traj_lens"], BC=BC, T=L)
    dyn = {"dynbuf": dynbuf}

    if "static_dev" in _CACHE:
        # optimistic: dispatch with the cached weight tensors, verify the
        # fingerprint while the device runs, re-run only on a (rare) change
        out_arrs = _dispatch(rt, dyn)
        digest = _weights_digest(inputs)
        if _CACHE.get("static_digest") != digest:
            _upload_static(inputs, rt, BC)
            _CACHE["static_digest"] = digest
            out_arrs = _dispatch(rt, dyn)
    else:
        digest = _weights_digest(inputs)
        _upload_static(inputs, rt, BC)
        _CACHE["static_digest"] = digest
        out_arrs = _dispatch(rt, dyn)

    res = np.asarray(out_arrs[rt["out_names"].index("out")], np.float32)
    res = res.reshape(NCORES, BC, C)
    out = np.empty((B, C), np.float32)
    for c in range(NCORES):
        out[c * BC + orders[c]] = res[c]
    return out



# revision 19
# speedup vs baseline: 1.3125x; 1.0333x over previous
"""Trainium2 Bass kernel for nn_ClassificationRNN2 (embedding + LSTM + ragged attention + head).

Strategy: data-parallel over batch across 8 NeuronCores (64 samples/core),
weights/embedding replicated, no collectives.

Host/runtime: the axon tunnel runs at ~35MB/s, so the warm-call cost is all
input transfer. The jitted shard_map executable is cached across calls and
the weight/embedding tensors live device-resident (refreshed only when a
content fingerprint changes, verified concurrently with the dispatch). Each
call ships one small i32 buffer per core (token tiles + sorted lens, ~77KB);
all ragged-length-dependent addressing (q/M gather offsets, softmax mask) is
derived from lens on device in exact f32.

Per-core layout: "transposed" H-major state. Per step t:
  g^T[1024,64] = Wcat^T.T @ [x_t; h_{t-1}]  (24 bf16 matmuls, fp32 PSUM)
  gates on ACT (sigmoid/tanh share one table set), cell update on DVE,
  h_t transposed (PE) to b-major and stored to a DRAM scratch [BC,T,H].
Attention reads that scratch: q via indirect gather at len-1, the ragged
reshape-view M[b] = flat_b.reshape(H, len_b) via indirect gather with
offsets h*len_b, score/ctx via per-sample matmuls.
"""

import numpy as np
import ml_dtypes

import concourse.bass as bass
import concourse.mybir as mybir
import concourse.tile as tile
from concourse import bacc
from concourse.bass import IndirectOffsetOnAxis
from concourse.masks import make_identity

BF16 = mybir.dt.bfloat16
F32 = mybir.dt.float32
I32 = mybir.dt.int32
AF = mybir.ActivationFunctionType
ALU = mybir.AluOpType
AX = mybir.AxisListType

NCORES = 8
B, L, D, H, V, C = 512, 300, 128, 256, 100001, 14
G = 4 * H  # 1024 gate dims


def build_kernel(BC, T, VV, CH_STEPS, enable_asserts=False):
    """Per-core program. BC=batch/core, T=steps, VV=vocab rows,
    CH_STEPS*BC must be a multiple of 128 and divide BC*T."""
    TOK = BC * T
    TOK_CH = BC * CH_STEPS
    assert TOK_CH % 128 == 0 and TOK % TOK_CH == 0
    TPC = TOK_CH // 128          # 128-token transpose tiles per chunk
    NCH = TOK // TOK_CH          # x^T chunks
    PW = ((T + 127) // 128) * 128
    NK = PW // 128               # l-chunks for ctx
    LCH = [min(128, T - k * 128) for k in range(NK)]

    nc = bacc.Bacc("TRN2", target_bir_lowering=False, debug=False,
                   enable_asserts=enable_asserts)

    # ---- DRAM I/O ----
    emb_d = nc.dram_tensor("emb", [VV, D], BF16, kind="ExternalInput")
    wt_d = nc.dram_tensor("wt", [3, 128, G], BF16, kind="ExternalInput")
    w1t_d = nc.dram_tensor("w1t", [4, 128, H], BF16, kind="ExternalInput")
    w2t_d = nc.dram_tensor("w2t", [2, 128, C], BF16, kind="ExternalInput")
    biasg_d = nc.dram_tensor("biasg", [128, 8], F32, kind="ExternalInput")
    b1t_d = nc.dram_tensor("b1t", [128, 2], F32, kind="ExternalInput")
    b2c_d = nc.dram_tensor("b2c", [C, 1], F32, kind="ExternalInput")
    NT = TOK // 128              # total 128-token tiles
    NW = (NT * 17 + 31) // 32    # 17-bit packed words per partition
    # single per-call tensor: cols 0..NW-1 = 17-bit-packed tokens (LSB-first
    # per partition), col NW = sorted lens
    dynbuf_d = nc.dram_tensor("dynbuf", [128, NW + 1], I32, kind="ExternalInput")
    # static iota/address-helper constants (uploaded once with the weights)
    iotal_d = nc.dram_tensor("iotal", [BC, T], F32, kind="ExternalInput")
    bth_d = nc.dram_tensor("bth", [128, BC], F32, kind="ExternalInput")
    iotap_d = nc.dram_tensor("iotap", [128, 2], F32, kind="ExternalInput")
    qbias_d = nc.dram_tensor("qbias", [BC, 1], F32, kind="ExternalInput")
    eye_d = nc.dram_tensor("eye", [1, BC * BC], F32, kind="ExternalInput")
    out_d = nc.dram_tensor("out", [BC, C], F32, kind="ExternalOutput")
    # internal DRAM scratch: per-sample row-major hidden states, flat for gathers
    hs_d = nc.dram_tensor("hsflat", [BC * T * H, 1], BF16)
    hs3 = hs_d[:].rearrange("(b t h) one -> b t (h one)", b=BC, t=T)

    with tile.TileContext(nc) as tc:
        with tc.tile_pool(name="persist", bufs=1) as pp:
            # ---- persistent SBUF ----
            idf = pp.tile([128, 128], F32, tag="idf")
            make_identity(nc, idf[:])
            idb = pp.tile([128, 128], BF16, tag="idb")
            nc.vector.tensor_copy(idb[:], idf[:])

            w_sb = pp.tile([128, 3 * G], BF16, tag="w")
            w1_sb = pp.tile([128, 4 * H], BF16, tag="w1")
            w2_sb = pp.tile([128, 2 * C], BF16, tag="w2")
            for k in range(3):
                nc.sync.dma_start(w_sb[:, k * G:(k + 1) * G], wt_d[k])
            for k in range(4):
                nc.sync.dma_start(w1_sb[:, k * H:(k + 1) * H], w1t_d[k])
            for k in range(2):
                nc.sync.dma_start(w2_sb[:, k * C:(k + 1) * C], w2t_d[k])
            bg_sb = pp.tile([128, 8], F32, tag="bg")
            nc.sync.dma_start(bg_sb[:], biasg_d[:])
            b1_sb = pp.tile([128, 2], F32, tag="b1")
            nc.sync.dma_start(b1_sb[:], b1t_d[:])
            b2_sb = pp.tile([C, 1], F32, tag="b2")
            nc.sync.dma_start(b2_sb[:], b2c_d[:])

            xT = [pp.tile([128, TOK_CH], BF16, tag=f"xT{c}", name=f"xT{c}")
                  for c in range(NCH)]

            # ========== phase 0: token unpack + ragged address math ==========
            # dynbuf carries 17-bit-packed tokens + sorted lens; unpack the
            # tokens (per-column immediate shifts: word/shift depend only on
            # the column) and derive qoff/moff/mask (f32 exact: all < 2^24).
            pk = pp.tile([128, NW + 1], I32, tag="pk")
            nc.sync.dma_start(pk[:], dynbuf_d[:])
            idx_all = pp.tile([128, NT], I32, tag="idx")
            tu1 = pp.tile([128, 1], I32, tag="tu1")
            tu2 = pp.tile([128, 1], I32, tag="tu2")
            for g in range(NT):
                w0, s = (17 * g) >> 5, (17 * g) & 31
                if s + 17 <= 32:
                    nc.vector.tensor_scalar(idx_all[:, g:g + 1], pk[:, w0:w0 + 1],
                                            s, scalar2=0x1FFFF,
                                            op0=ALU.logical_shift_right,
                                            op1=ALU.bitwise_and)
                else:
                    nc.vector.tensor_scalar(tu1[:], pk[:, w0:w0 + 1], s,
                                            scalar2=None,
                                            op0=ALU.logical_shift_right)
                    nc.vector.tensor_scalar(tu2[:], pk[:, w0 + 1:w0 + 2], 32 - s,
                                            scalar2=None,
                                            op0=ALU.logical_shift_left)
                    nc.vector.tensor_tensor(out=tu2[:], in0=tu2[:], in1=tu1[:],
                                            op=ALU.bitwise_or)
                    nc.vector.tensor_scalar(idx_all[:, g:g + 1], tu2[:], 0x1FFFF,
                                            scalar2=None, op0=ALU.bitwise_and)
            iotal_sb = pp.tile([BC, T], F32, tag="iotal")
            nc.sync.dma_start(iotal_sb[:], iotal_d[:])
            bth_sb = pp.tile([128, BC], F32, tag="bth")
            nc.sync.dma_start(bth_sb[:], bth_d[:])
            iotap_sb = pp.tile([128, 2], F32, tag="iotap")
            nc.sync.dma_start(iotap_sb[:], iotap_d[:])
            qbias_sb = pp.tile([BC, 1], F32, tag="qbias")
            nc.sync.dma_start(qbias_sb[:], qbias_d[:])
            ones_sb = pp.tile([1, 128], F32, tag="ones")
            nc.vector.memset(ones_sb[:], 1.0)

            lens_f = pp.tile([BC, 1], F32, tag="lensf")
            nc.vector.tensor_copy(lens_f[:], pk[:BC, NW:NW + 1])
            moff = pp.tile([128, 2 * BC], I32, tag="moff")
            qoff = pp.tile([BC, 1], I32, tag="qoff")
            mask = pp.tile([BC, T], F32, tag="mask")
            qf = pp.tile([BC, 1], F32, tag="qf")
            # qoff = lens*H + (b*T*H - H)
            nc.vector.tensor_scalar(qf[:], lens_f[:], float(H),
                                    scalar2=qbias_sb[:, 0:1],
                                    op0=ALU.mult, op1=ALU.add)
            nc.vector.tensor_copy(qoff[:], qf[:])
            # mask = (l >= lens) * -1e30
            nc.vector.tensor_scalar(mask[:], iotal_sb[:], lens_f[:, 0:1],
                                    scalar2=-1e30,
                                    op0=ALU.is_ge, op1=ALU.mult)
            with tc.tile_pool(name="p0", bufs=1, space="PSUM") as ps0:
                # lens broadcast over partitions: lensB[p,b] = lens[b]
                lrp = ps0.tile([1, BC], F32, tag="lrp")
                nc.tensor.matmul(out=lrp[:], lhsT=lens_f[:], rhs=idf[:BC, :BC],
                                 start=True, stop=True)
                lens_row = pp.tile([1, BC], F32, tag="lensrow")
                nc.vector.tensor_copy(lens_row[:], lrp[:])
                lBp = ps0.tile([128, BC], F32, tag="lBp")
                nc.tensor.matmul(out=lBp[:], lhsT=ones_sb[:], rhs=lens_row[:],
                                 start=True, stop=True)
                lensB = pp.tile([128, BC], F32, tag="lensB")
                nc.vector.tensor_copy(lensB[:], lBp[:])
            # moff[u][p,b] = (u*128+p)*lens[b] + b*T*H
            mf = pp.tile([128, 2 * BC], F32, tag="mf")
            for u in range(2):
                cs = slice(u * BC, (u + 1) * BC)
                nc.vector.tensor_scalar(mf[:, cs], lensB[:], iotap_sb[:, u:u + 1],
                                        scalar2=None, op0=ALU.mult)
                nc.vector.tensor_tensor(out=mf[:, cs], in0=mf[:, cs],
                                        in1=bth_sb[:], op=ALU.add)
            nc.vector.tensor_copy(moff[:], mf[:])

            # ========== phase 1: embedding gather + transpose to x^T ==========
            # HW indirect DMA consumes ONE offset per dest partition; the whole
            # per-partition free extent streams contiguously from it. So gather
            # one 128-token tile ([128, D]) per instruction.
            with tc.tile_pool(name="gat", bufs=4) as gp, \
                 tc.tile_pool(name="ps1", bufs=2, space="PSUM") as ps1:
                for g in range(NT):
                    ci, j = g // TPC, g % TPC
                    xrows = gp.tile([128, 128], BF16, tag="xrows")
                    nc.gpsimd.indirect_dma_start(
                        out=xrows[:], out_offset=None,
                        in_=emb_d[:],
                        in_offset=IndirectOffsetOnAxis(ap=idx_all[:, g:g + 1],
                                                       axis=0),
                    )
                    trp = ps1.tile([128, 128], BF16, tag="trx")
                    nc.tensor.transpose(out=trp[:], in_=xrows[:],
                                        identity=idb[:])
                    nc.vector.tensor_copy(
                        xT[ci][:, j * 128:(j + 1) * 128], trp[:])

                # ========== phase 2: LSTM recurrence ==========
                with tc.tile_pool(name="st", bufs=1) as sp, \
                     tc.tile_pool(name="lp", bufs=2) as lp, \
                     tc.tile_pool(name="ps2", bufs=2, space="PSUM") as ps2:
                    c_sb = sp.tile([128, 2 * BC], F32, tag="c")
                    nc.gpsimd.memset(c_sb[:], 0.0)
                    hT_prev = lp.tile([128, 2 * BC], BF16, tag="hT")
                    nc.gpsimd.memset(hT_prev[:], 0.0)

                    for t in range(T):
                        ch, col = t // CH_STEPS, (t % CH_STEPS) * BC
                        xcol = xT[ch][:, col:col + BC]
                        gA = ps2.tile([128, 4 * BC], F32, tag="gA")
                        gB = ps2.tile([128, 4 * BC], F32, tag="gB")
                        for j in range(8):
                            out = (gA if j < 4 else gB)[:, (j % 4) * BC:(j % 4 + 1) * BC]
                            wj = slice(j * 128, (j + 1) * 128)
                            nc.tensor.matmul(out=out, lhsT=w_sb[:, wj], rhs=xcol,
                                             start=True, stop=False)
                            nc.tensor.matmul(out=out, lhsT=w_sb[:, G:][:, wj],
                                             rhs=hT_prev[:, :BC], start=False, stop=False)
                            nc.tensor.matmul(out=out, lhsT=w_sb[:, 2 * G:][:, wj],
                                             rhs=hT_prev[:, BC:], start=False, stop=True)
                        # gates: i=j0,1  f=j2,3 (gA)   g~=j4,5  o=j6,7 (gB)
                        i_sb = lp.tile([128, 2 * BC], F32, tag="i")
                        f_sb = lp.tile([128, 2 * BC], F32, tag="f")
                        g_sb = lp.tile([128, 2 * BC], F32, tag="g")
                        o_sb = lp.tile([128, 2 * BC], F32, tag="o")
                        for u in range(2):
                            cs = slice(u * BC, (u + 1) * BC)
                            cs2 = slice(2 * BC + u * BC, 2 * BC + (u + 1) * BC)
                            nc.scalar.activation(i_sb[:, cs], gA[:, cs], AF.Sigmoid,
                                                 bias=bg_sb[:, u:u + 1])
                            nc.scalar.activation(f_sb[:, cs], gA[:, cs2], AF.Sigmoid,
                                                 bias=bg_sb[:, 2 + u:3 + u])
                            nc.scalar.activation(g_sb[:, cs], gB[:, cs], AF.Tanh,
                                                 bias=bg_sb[:, 4 + u:5 + u])
                            nc.scalar.activation(o_sb[:, cs], gB[:, cs2], AF.Sigmoid,
                                                 bias=bg_sb[:, 6 + u:7 + u])
                        t1 = lp.tile([128, 2 * BC], F32, tag="t1")
                        nc.vector.tensor_tensor(out=t1[:], in0=i_sb[:], in1=g_sb[:],
                                                op=ALU.mult)
                        nc.vector.tensor_tensor(out=c_sb[:], in0=c_sb[:], in1=f_sb[:],
                                                op=ALU.mult)
                        nc.vector.tensor_tensor(out=c_sb[:], in0=c_sb[:], in1=t1[:],
                                                op=ALU.add)
                        th = lp.tile([128, 2 * BC], F32, tag="th")
                        nc.scalar.activation(th[:], c_sb[:], AF.Tanh)
                        hT = lp.tile([128, 2 * BC], BF16, tag="hT")
                        nc.vector.tensor_tensor(out=hT[:], in0=o_sb[:], in1=th[:],
                                                op=ALU.mult)
                        # b-major row to DRAM for the attention phase
                        hrow = lp.tile([BC, H], BF16, tag="hrow")
                        for u in range(2):
                            trh = ps2.tile([BC, 128], BF16, tag="trh")
                            nc.tensor.transpose(out=trh[:],
                                                in_=hT[:, u * BC:(u + 1) * BC],
                                                identity=idb[:])
                            nc.vector.tensor_copy(hrow[:, u * 128:(u + 1) * 128],
                                                  trh[:])
                        nc.sync.dma_start(hs3[:, t, :], hrow[:])
                        hT_prev = hT

            # ========== phase 3: ragged attention + classifier head ==========
            with tc.tile_pool(name="at", bufs=1) as at, \
                 tc.tile_pool(name="ab", bufs=4) as ab, \
                 tc.tile_pool(name="ps3", bufs=2, space="PSUM") as ps3, \
                 tc.tile_pool(name="ps4", bufs=1, space="PSUM") as ps4:
                # M: per sample the reshape-view [H, len_b] padded to T cols
                # (moff precomputed on device in phase 0)
                Mt = [at.tile([128, BC * T], BF16, tag=f"Mt{u}", name=f"Mt{u}")
                      for u in range(2)]
                # b-major issue order: with samples sorted shortest-first,
                # gather b fires as soon as the stores for steps <= lens[b]
                # land, overlapping the remaining recurrence.
                for b in range(BC):
                    for u in range(2):
                        nc.gpsimd.indirect_dma_start(
                            out=Mt[u][:, b * T:(b + 1) * T], out_offset=None,
                            in_=hs_d[:],
                            in_offset=IndirectOffsetOnAxis(
                                ap=moff[:, u * BC + b:u * BC + b + 1], axis=0))

                # q = h[len-1] per sample -> qT [128, BC] x2 (bf16). Issued AFTER
                # the M gathers: q depends on the longest sample's last
                # store, and the gpsimd queue is in-order - putting it
                # first would head-of-line block all M gathers.
                qrow = at.tile([BC, H], BF16, tag="qrow")
                nc.gpsimd.indirect_dma_start(
                    out=qrow[:], out_offset=None, in_=hs_d[:],
                    in_offset=IndirectOffsetOnAxis(ap=qoff[:], axis=0))
                qT = at.tile([128, 2 * BC], BF16, tag="qT")
                for u in range(2):
                    trq = ps3.tile([128, BC], BF16, tag="tr")
                    nc.tensor.transpose(out=trq[:],
                                        in_=qrow[:, u * 128:(u + 1) * 128],
                                        identity=idb[:BC, :BC])
                    nc.vector.tensor_copy(qT[:, u * BC:(u + 1) * BC], trq[:])

                # scores: per sample q_b . M_b -> [1, T] row, then rank-1
                # accumulate rows into a [BC, T] PSUM via one-hot columns
                eye_sb = at.tile([1, BC * BC], F32, tag="eye")
                nc.sync.dma_start(eye_sb[:], eye_d[:])
                score_ps = ps4.tile([BC, T], F32, tag="scoreacc")
                for b in range(BC):
                    scp = ps3.tile([1, T], F32, tag="sc")
                    nc.tensor.matmul(out=scp[:], lhsT=qT[:, b:b + 1],
                                     rhs=Mt[0][:, b * T:(b + 1) * T],
                                     start=True, stop=False)
                    nc.tensor.matmul(out=scp[:], lhsT=qT[:, BC + b:BC + b + 1],
                                     rhs=Mt[1][:, b * T:(b + 1) * T],
                                     start=False, stop=True)
                    rsb = ab.tile([1, T], F32, tag="rsb")
                    nc.scalar.copy(rsb[:], scp[:])
                    nc.tensor.matmul(out=score_ps[:],
                                     lhsT=eye_sb[0:1, b * BC:(b + 1) * BC],
                                     rhs=rsb[:], start=(b == 0), stop=(b == BC - 1))
                score = at.tile([BC, T], F32, tag="score")
                nc.vector.tensor_copy(score[:], score_ps[:])
                nc.vector.tensor_tensor(out=score[:], in0=score[:], in1=mask[:],
                                        op=ALU.add)
                # softmax over T (free dim)
                mx = at.tile([BC, 1], F32, tag="mx")
                nc.vector.tensor_reduce(mx[:], score[:], axis=AX.X, op=ALU.max,
                                        negate=True)
                prob = at.tile([BC, PW], F32, tag="prob")
                nc.gpsimd.memset(prob[:], 0.0)
                sm = at.tile([BC, 1], F32, tag="sm")
                nc.scalar.activation(prob[:, :T], score[:], AF.Exp,
                                     bias=mx[:, 0:1], accum_out=sm[:, 0:1])
                rs = at.tile([BC, 1], F32, tag="rs")
                nc.vector.reciprocal(rs[:], sm[:])
                nc.vector.tensor_scalar_mul(prob[:, :T], prob[:, :T], rs[:, 0:1])
                # prob^T in bf16, [128, NK*BC]
                pT = at.tile([128, NK * BC], BF16, tag="pT")
                for k in range(NK):
                    trp2 = ps3.tile([128, BC], F32, tag="tr")
                    nc.tensor.transpose(out=trp2[:],
                                        in_=prob[:, k * 128:(k + 1) * 128],
                                        identity=idf[:BC, :BC])
                    nc.vector.tensor_copy(pT[:, k * BC:(k + 1) * BC], trp2[:])

                # ctx^T [H, BC]: per sample sum_l prob[l] * hs_b[l, :]
                ctxp = [ps4.tile([128, BC], F32, tag=f"ctx{u}", name=f"ctx{u}")
                        for u in range(2)]
                for b in range(BC):
                    ob = ab.tile([128, NK * H], BF16, tag="ob")
                    for k, lk in enumerate(LCH):
                        nc.sync.dma_start(ob[:lk, k * H:k * H + H],
                                          hs3[b, k * 128:k * 128 + lk, :])
                    for u in range(2):
                        for k, lk in enumerate(LCH):
                            nc.tensor.matmul(
                                out=ctxp[u][:, b:b + 1],
                                lhsT=ob[:lk, k * H + u * 128:k * H + (u + 1) * 128],
                                rhs=pT[:lk, k * BC + b:k * BC + b + 1],
                                start=(k == 0), stop=(k == NK - 1),
                                skip_group_check=True)
                ctxT = at.tile([128, 2 * BC], BF16, tag="ctxT")
                for u in range(2):
                    nc.vector.tensor_copy(ctxT[:, u * BC:(u + 1) * BC], ctxp[u][:])

                # a^T = tanh(W1 @ [ctx; q] + b1)  [H, BC]
                rhs4 = [ctxT[:, :BC], ctxT[:, BC:], qT[:, :BC], qT[:, BC:]]
                aT = at.tile([128, 2 * BC], BF16, tag="aT")
                for m in range(2):
                    atp = ps4.tile([128, BC], F32, tag="atp")
                    for k in range(4):
                        nc.tensor.matmul(
                            out=atp[:],
                            lhsT=w1_sb[:, k * H + m * 128:k * H + (m + 1) * 128],
                            rhs=rhs4[k], start=(k == 0), stop=(k == 3))
                    nc.scalar.activation(aT[:, m * BC:(m + 1) * BC], atp[:], AF.Tanh,
                                         bias=b1_sb[:, m:m + 1])
                # logits^T [C, BC] + b2; transpose; softmax over C
                lgp = ps3.tile([C, BC], F32, tag="tr")
                nc.tensor.matmul(out=lgp[:], lhsT=w2_sb[:, :C], rhs=aT[:, :BC],
                                 start=True, stop=False)
                nc.tensor.matmul(out=lgp[:], lhsT=w2_sb[:, C:], rhs=aT[:, BC:],
                                 start=False, stop=True)
                lg = at.tile([C, BC], F32, tag="lg")
                nc.scalar.activation(lg[:], lgp[:], AF.Identity, bias=b2_sb[:, 0:1])
                lgTp = ps3.tile([BC, C], F32, tag="tr")
                nc.tensor.transpose(out=lgTp[:], in_=lg[:], identity=idf[:C, :C])
                lgT = at.tile([BC, C], F32, tag="lgT")
                nc.vector.tensor_copy(lgT[:], lgTp[:])
                mx2 = at.tile([BC, 1], F32, tag="mx2")
                nc.vector.tensor_reduce(mx2[:], lgT[:], axis=AX.X, op=ALU.max,
                                        negate=True)
                sm2 = at.tile([BC, 1], F32, tag="sm2")
                pr2 = at.tile([BC, C], F32, tag="pr2")
                nc.scalar.activation(pr2[:], lgT[:], AF.Exp, bias=mx2[:, 0:1],
                                     accum_out=sm2[:, 0:1])
                rs2 = at.tile([BC, 1], F32, tag="rs2")
                nc.vector.reciprocal(rs2[:], sm2[:])
                nc.vector.tensor_scalar_mul(pr2[:], pr2[:], rs2[:, 0:1])
                nc.sync.dma_start(out_d[:], pr2[:])
    nc.compile()
    return nc


def static_prep(emb, W_ih, W_hh, b_ih, b_hh, W1, b1, W2, b2, BC):
    """Weight/table tensors — identical for every core and (in practice)
    every call. Uploaded to device once and cached."""
    bf = ml_dtypes.bfloat16
    Hh, Cc, Gg = H, C, G
    emb_bf = np.ascontiguousarray(np.asarray(emb, np.float32).astype(bf))
    Wcat = np.concatenate([np.asarray(W_ih, np.float32),
                           np.asarray(W_hh, np.float32)], axis=1)  # [G, D+H]
    wt = np.ascontiguousarray(Wcat.T.astype(bf)).reshape(3, 128, Gg)
    w1t = np.ascontiguousarray(np.asarray(W1, np.float32).T.astype(bf)).reshape(4, 128, Hh)
    w2t = np.ascontiguousarray(np.asarray(W2, np.float32).T.astype(bf)).reshape(2, 128, Cc)
    biasg = np.ascontiguousarray(
        (np.asarray(b_ih, np.float32) + np.asarray(b_hh, np.float32))
        .reshape(8, 128).T.astype(np.float32))
    b1t = np.ascontiguousarray(np.asarray(b1, np.float32).reshape(2, 128).T)
    b2c = np.ascontiguousarray(np.asarray(b2, np.float32).reshape(Cc, 1))
    eye = np.ascontiguousarray(np.eye(BC, dtype=np.float32).reshape(1, BC * BC))
    T = L
    iotal = np.broadcast_to(np.arange(T, dtype=np.float32), (BC, T)).copy()
    bth = np.broadcast_to((np.arange(BC) * T * Hh).astype(np.float32), (128, BC)).copy()
    iotap = np.stack([np.arange(128, dtype=np.float32),
                      np.arange(128, dtype=np.float32) + 128.0], axis=1)
    qbias = (np.arange(BC, dtype=np.float32) * T * Hh - Hh).reshape(BC, 1)
    return dict(emb=emb_bf, wt=wt, w1t=w1t, w2t=w2t, biasg=biasg, b1t=b1t,
                b2c=b2c, eye=eye, iotal=iotal, bth=bth,
                iotap=np.ascontiguousarray(iotap), qbias=qbias)


def _pack17(tok):
    """tok: [128, NT] int32 (<2^17) -> [128, NW] int32, 17-bit LSB-first
    bitstream per partition row."""
    NT = tok.shape[1]
    NW = (NT * 17 + 31) // 32
    t = np.concatenate([tok.astype(np.uint64),
                        np.zeros((tok.shape[0], 3), np.uint64)], axis=1)
    A = (32 * np.arange(NW)) // 17               # first token index in word w
    S = (32 * np.arange(NW) - 17 * A).astype(np.uint64)
    w = t[:, A] >> S
    w |= t[:, A + 1] << (17 - S)
    sh2 = 34 - S
    w |= np.where(sh2 < 64, t[:, A + 2] << np.minimum(sh2, np.uint64(63)), 0)
    return (w & 0xFFFFFFFF).astype(np.uint32).view(np.int32)


def dynamic_prep(inputs_arrays, traj_lens, BC, T):
    """Per-call tensor: 17-bit-packed token tiles + sorted lens in one i32
    buffer (~41KB/core), built directly in global (8*128, NW+1) layout."""
    n_cores = np.asarray(inputs_arrays).shape[0] // BC
    idx_all = np.asarray(inputs_arrays).astype(np.int64)
    lens_all = np.asarray(traj_lens).astype(np.int64)
    NT = BC * T // 128
    NW = (NT * 17 + 31) // 32
    dynbuf = np.zeros((n_cores * 128, NW + 1), np.int32)
    orders = []
    for c in range(n_cores):
        idx = idx_all[c * BC:(c + 1) * BC]          # [BC, T]
        lens = lens_all[c * BC:(c + 1) * BC]        # [BC]
        # Sort samples shortest-first within the core: M-gather slot b then
        # depends only on recurrence steps <= lens[b], so the in-order gpsimd
        # queue drains progressively during the recurrence instead of
        # serializing after it. Rows are un-permuted host-side in kernel().
        order = np.argsort(lens, kind="stable")
        orders.append(order)
        idx = idx[order]
        lens = lens[order]
        tok = idx.T.reshape(-1)                     # t-major token order
        blk = dynbuf[c * 128:(c + 1) * 128]
        blk[:, :NW] = _pack17(tok.reshape(NT, 128).astype(np.int32).T)
        blk[:BC, NW] = lens.astype(np.int32)
    return dynbuf, orders


_CACHE = {}


def _weights_digest(inputs):
    """Cheap content fingerprint of the weight tensors so cached device
    copies are refreshed if a caller ever changes them between calls."""
    parts = []
    for k in ("emb", "W_ih", "W_hh", "b_ih", "b_hh", "W1", "b1", "W2", "b2"):
        a = np.ascontiguousarray(np.asarray(inputs[k]))
        if a.nbytes % 8 == 0:
            v = a.reshape(-1).view(np.uint64)
        elif a.nbytes % 4 == 0:
            v = a.reshape(-1).view(np.uint32)
        else:
            v = a.reshape(-1).view(np.uint8)
        parts.append((a.shape, str(a.dtype), int(np.sum(v, dtype=np.uint64))))
    return tuple(parts)


def _get_runtime():
    """Build the Bass program once and wrap it in a cached jitted shard_map
    (mirrors concourse.bass2jax.run_bass_via_pjrt, but reusable across calls
    so compiled executable + device-resident weights are not re-shipped)."""
    if "rt" in _CACHE:
        return _CACHE["rt"]
    import jax
    from jax.experimental.shard_map import shard_map
    from jax.sharding import Mesh, NamedSharding, PartitionSpec
    from concourse.bass2jax import (_bass_exec_p, install_neuronx_cc_hook,
                                    partition_id_tensor)

    install_neuronx_cc_hook()
    nc = build_kernel(BC=B // NCORES, T=L, VV=V, CH_STEPS=20)
    partition_name = (nc.partition_id_tensor.name
                      if nc.partition_id_tensor else None)
    in_names, out_names, out_avals, zero_outs = [], [], [], []
    for alloc in nc.m.functions[0].allocations:
        if not isinstance(alloc, mybir.MemoryLocationSet):
            continue
        name = alloc.memorylocations[0].name
        if alloc.kind == "ExternalInput":
            if name != partition_name:
                in_names.append(name)
        elif alloc.kind == "ExternalOutput":
            out_names.append(name)
            shape = tuple(alloc.tensor_shape)
            dtype = mybir.dt.np(alloc.dtype)
            out_avals.append(jax.core.ShapedArray(shape, dtype))
            zero_outs.append(np.zeros((NCORES * shape[0], *shape[1:]), dtype))
    n_params = len(in_names)
    n_outs = len(out_names)
    in_names_all = list(in_names) + list(out_names)
    if partition_name is not None:
        in_names_all.append(partition_name)
    donate = tuple(range(n_params, n_params + n_outs))

    def _body(*args):
        operands = list(args)
        if partition_name is not None:
            operands.append(partition_id_tensor())
        outs = _bass_exec_p.bind(
            *operands, out_avals=tuple(out_avals), in_names=tuple(in_names_all),
            out_names=tuple(out_names), lowering_input_output_aliases=(),
            sim_require_finite=True, sim_require_nnan=True, nc=nc)
        return tuple(outs)

    devices = jax.devices()[:NCORES]
    assert len(devices) == NCORES
    mesh = Mesh(np.asarray(devices), ("core",))
    in_specs = (PartitionSpec("core"),) * (n_params + n_outs)
    out_specs = (PartitionSpec("core"),) * n_outs
    sharded = jax.jit(
        shard_map(_body, mesh=mesh, in_specs=in_specs, out_specs=out_specs,
                  check_rep=False),
        donate_argnums=donate, keep_unused=True)
    rt = dict(sharded=sharded, in_names=in_names, out_names=out_names,
              zero_outs=zero_outs,
              sharding=NamedSharding(mesh, PartitionSpec("core")))
    _CACHE["rt"] = rt
    return rt


def _upload_static(inputs, rt, BC):
    import jax
    st = static_prep(inputs["emb"], inputs["W_ih"], inputs["W_hh"],
                     inputs["b_ih"], inputs["b_hh"], inputs["W1"],
                     inputs["b1"], inputs["W2"], inputs["b2"], BC=BC)
    _CACHE["static_dev"] = {
        k: jax.device_put(np.concatenate([v] * NCORES, axis=0),
                          rt["sharding"])
        for k, v in st.items()}


def _dispatch(rt, dyn):
    args = [_CACHE["static_dev"].get(nm, dyn.get(nm))
            for nm in rt["in_names"]] + list(rt["zero_outs"])
    return rt["sharded"](*args)


def kernel(**inputs):
    BC = B // NCORES
    rt = _get_runtime()

    dynbuf, orders = dynamic_prep(inputs["inputs_arrays"],
                                  inputs["traj_lens"], BC=BC, T=L)
    dyn = {"dynbuf": dynbuf}

    if "static_dev" in _CACHE:
        # optimistic: dispatch with the cached weight tensors, verify the
        # fingerprint while the device runs, re-run only on a (rare) change
        out_arrs = _dispatch(rt, dyn)
        digest = _weights_digest(inputs)
        if _CACHE.get("static_digest") != digest:
            _upload_static(inputs, rt, BC)
            _CACHE["static_digest"] = digest
            out_arrs = _dispatch(rt, dyn)
    else:
        digest = _weights_digest(inputs)
        _upload_static(inputs, rt, BC)
        _CACHE["static_digest"] = digest
        out_arrs = _dispatch(rt, dyn)

    res = np.asarray(out_arrs[rt["out_names"].index("out")], np.float32)
    res = res.reshape(NCORES, BC, C)
    out = np.empty((B, C), np.float32)
    for c in range(NCORES):
        out[c * BC + orders[c]] = res[c]
    return out

